# revision 9
# baseline (speedup 1.0000x reference)
"""GQA kernel for Trainium2, sharded over 8 NeuronCores.

Sharding: data-parallel over batch (2) x tensor-parallel over kv_heads (4).
Core c = b*4 + h computes the full attention output partial
    Y_bh = softmax(causal((Q_b @ Wq_eff_h) @ (K_b @ Wk_h)^T / sqrt(dk))) @ (V_b @ Wv_h) @ Wo_h
and the host sums the 4 head partials per batch (the "all-reduce after Wo").

The GQA group-sum-before-softmax quirk folds into the weights:
    scores_h = sum_g (Q Wq_{g,h}) (K Wk_h)^T = (Q [sum_g Wq_{g,h}]) (K Wk_h)^T
so Wq_eff_h = sum_g Wq[:, (g*KV+h)*dk : ...] and each core runs standard attention.

Schedule (DMA wire ~360GB/s is the binding resource; PE work ~ wire time):
  A: kt row-stream -> k projection (warm fills keep the PE p-state high).
  B: qt COLUMN-blocked stream (host pre-blocks qt so each 512-query block
     is contiguous): qT_j completes per block, so scores for chunk j
     (S^T tiles -> exp -> causal mask) run DURING the q stream. Rowsum
     ones-matmuls for early chunks fill B's leftover PE slack.
  C: vt row-stream -> v projection + remaining rowsum ones-matmuls
     (always-ready PE work packs the DMA-wait gaps).
  D: per key-block v eviction/transpose feeding PV accumulation, softmax
     normalization folded into O^T eviction, Y = O @ Wo, writes issued
     per 128-row block so the 8MB output stream starts immediately and
     runs back-to-back behind the PE.

Layouts (SBUF partition dim first): qT/kT/vT (dk=128, L) fp16; S^T tiles
(Lk_t=128, Lq=512) fp32 psum; row sums via ones-matmul (result replicated
across partitions == the free-dim broadcast needed to normalize O^T).
PSUM budget: acc pool (4 banks: kaccs in A, vaccs in C) + rot pool
(3 banks: qacc/st/rrep in B-C, tp/ot/yps in D) + warm (1) = 8 banks.
"""
import sys
sys.path.insert(0, '/opt/trn_rl_repo')
import math
import numpy as np

import concourse.bass as bass
import concourse.mybir as mybir
import concourse.tile as tile
from concourse import bacc
from concourse import bass_utils
from concourse.masks import make_identity

FP32 = mybir.dt.float32
FP16 = mybir.dt.float16

B, L, D = 2, 2048, 2048
Q_HEADS, KV_HEADS, DK, DV = 16, 4, 128, 128
GROUPS = Q_HEADS // KV_HEADS
P = 128
CH = 512                 # Lq chunk width
NJ = L // CH             # 4 query chunks
NDC = D // P             # 16 contraction chunks
NLK = L // P             # 16 key tiles
SCALE = 1.0 / math.sqrt(DK)
EBIAS = -8.0 * math.log(2.0)   # exp output scaled by 2^-8; cancels in softmax
YDT = FP16               # partial-output dtype (host accumulates in fp32)
YNP = np.float16

# score work item (j, c): S^T tile for query chunk j, key P-block c <= 4j+3
ET_OFF = {}
_off = 0
for _j in range(NJ):
    for _c in range(4 * _j + 4):
        ET_OFF[(_j, _c)] = _off
        _off += CH
ET_W = _off              # 40 * 512 fp16 = 40KB/partition


def _build():
    nc = bacc.Bacc(trn_type="TRN2")
    # qt is host-pre-blocked: row j*D+dc*128+p, col x  ==  qT-input for
    # query block j, contraction chunk dc  (each (128,512) tile contiguous)
    qt_d = nc.dram_tensor("qt", (NJ * D, CH), FP16, kind="ExternalInput")
    kt_d = nc.dram_tensor("kt", (D, L), FP16, kind="ExternalInput")
    vt_d = nc.dram_tensor("vt", (D, L), FP16, kind="ExternalInput")
    # weights pre-packed on host to the SBUF image: (128, NDC*dk)
    wq_d = nc.dram_tensor("wq", (P, NDC * DK), FP16, kind="ExternalInput")
    wk_d = nc.dram_tensor("wk", (P, NDC * DK), FP16, kind="ExternalInput")
    wv_d = nc.dram_tensor("wv", (P, NDC * DV), FP16, kind="ExternalInput")
    wo_d = nc.dram_tensor("wo", (DV, D), FP16, kind="ExternalInput")
    y_d = nc.dram_tensor("y", (L, D), YDT, kind="ExternalOutput")

    with tile.TileContext(nc) as tc:
        with (
            tc.tile_pool(name="const", bufs=1) as const,
            tc.tile_pool(name="wpool", bufs=1) as wpool,
            tc.tile_pool(name="xs", bufs=6) as xs,
            tc.tile_pool(name="xq", bufs=20) as xq,
            tc.tile_pool(name="proj", bufs=1) as proj,
            tc.tile_pool(name="ev", bufs=4) as ev_pool,
            tc.tile_pool(name="acc", bufs=4, space="PSUM") as acc_p,
            tc.tile_pool(name="rot", bufs=3, space="PSUM") as rot_p,
        ):
            ident = const.tile([P, P], FP16)
            make_identity(nc, ident[:])
            ones = const.tile([P, P], FP16)
            nc.vector.memset(ones[:], 1.0)
            ones2 = const.tile([P, 256], FP16)
            nc.vector.memset(ones2[:], 1.0)
            ebias = const.tile([P, 1], FP32)
            nc.vector.memset(ebias[:], EBIAS)

            # causal mask for the 4 diagonal tiles, built on-device:
            # maskt[p, d*CH + x] = (128*d + p <= x)
            maskt = const.tile([P, NJ * CH], FP16)
            nc.gpsimd.memset(maskt[:], 1.0)
            for d in range(4):
                nc.gpsimd.affine_select(
                    out=maskt[:, d * CH:(d + 1) * CH],
                    in_=maskt[:, d * CH:(d + 1) * CH],
                    compare_op=mybir.AluOpType.is_ge,
                    fill=0.0,
                    base=-(P * d),
                    channel_multiplier=-1,
                    pattern=[[1, CH]],
                )

            kT = proj.tile([P, L], FP16, tag="kT")
            qT = proj.tile([P, L], FP16, tag="qT")
            vT = proj.tile([P, L], FP16, tag="vT")
            v_nat = proj.tile([P, L], FP16, tag="v_nat")
            oT = proj.tile([P, L], FP16, tag="oT")
            et_all = proj.tile([P, ET_W], FP16, tag="et_all")
            rsum_all = proj.tile([P, NJ * CH], FP32, tag="rsum_all")
            rtot_all = proj.tile([P, NJ * CH], FP32, tag="rtot_all")
            rinv_all = proj.tile([P, NJ * CH], FP32, tag="rinv_all")

            w_sbs = {}

            def load_w(name, wd):
                w_sb = wpool.tile([P, NDC * DK], FP16, tag=name, name=name)
                nc.scalar.dma_start(w_sb[:], wd[:])
                w_sbs[name] = w_sb

            warm = rot_p.tile([P, CH], FP32, tag="warm", bufs=1, name="warm")

            # --- phase A: k projection (row-streamed) ---
            load_w("wk", wk_d)
            load_w("wq", wq_d)
            w_sb = w_sbs["wk"]
            kaccs = [acc_p.tile([P, CH], FP32, tag="acc", name=f"kacc{j}")
                     for j in range(NJ)]
            for dc in range(NDC):
                xt = xs.tile([P, L], FP16, tag="xt", name="xt")
                eng = (nc.sync, nc.gpsimd)[dc % 2] if dc < 6 else nc.sync
                eng.dma_start(xt[:], kt_d[dc * P:(dc + 1) * P, :])
                for j in range(NJ):
                    nc.tensor.matmul(
                        kaccs[j][:], w_sb[:, dc * P:dc * P + P],
                        xt[:, j * CH:(j + 1) * CH],
                        start=(dc == 0), stop=(dc == NDC - 1))
                # keep the PE p-state ramped while the wire streams
                nc.tensor.matmul(warm[:, 0:256], ones[:], ones2[:],
                                 start=True, stop=True)
            for j in range(NJ):
                nc.vector.tensor_copy(kT[:, j * CH:(j + 1) * CH], kaccs[j][:])

            # --- phase B: q blocked-stream; scores(j) during block j+1 ---
            load_w("wv", wv_d)
            wo_sb = wpool.tile([DV, D], FP16)
            nc.scalar.dma_start(wo_sb[:], wo_d[:])

            def score_st(j, c):
                st = rot_p.tile([P, CH], FP32, tag="rot", name="st")
                nc.tensor.matmul(st[:], kT[:, c * P:(c + 1) * P],
                                 qT[:, j * CH:(j + 1) * CH],
                                 start=True, stop=True)
                et = et_all[:, ET_OFF[(j, c)]:ET_OFF[(j, c)] + CH]
                nc.scalar.activation(et, st[:],
                                     mybir.ActivationFunctionType.Exp,
                                     bias=ebias[:], scale=SCALE)
                d = c - 4 * j
                if d >= 0:   # diagonal tile: zero out k > q
                    nc.vector.tensor_mul(et, et, maskt[:, d * CH:(d + 1) * CH])
                # rowsum accumulation on DVE: PE stays decoupled from the
                # scalar exp stream (a PE ones-matmul here would stall the
                # in-order PE queue on every exp)
                rs = rsum_all[:, j * CH:(j + 1) * CH]
                if c == 0:
                    nc.vector.tensor_copy(rs, et)
                else:
                    nc.vector.tensor_add(rs, rs, et)

            w_sb = w_sbs["wq"]
            qde = [nc.sync, nc.gpsimd]
            for j in range(NJ):
                qacc = rot_p.tile([P, CH], FP32, tag="rot", name="qacc")
                for dc in range(NDC):
                    if j == 0:
                        nc.tensor.matmul(warm[:, 0:256], ones[:], ones2[:],
                                         start=True, stop=True)
                    xt = xq.tile([P, CH], FP16, tag="xq", name="xq")
                    qde[dc % 2].dma_start(
                        xt[:], qt_d[j * D + dc * P:j * D + (dc + 1) * P, :])
                    nc.tensor.matmul(qacc[:], w_sb[:, dc * P:dc * P + P],
                                     xt[:], start=(dc == 0),
                                     stop=(dc == NDC - 1))
                nc.vector.tensor_copy(qT[:, j * CH:(j + 1) * CH], qacc[:])
                for c in range(4 * j + 4):
                    score_st(j, c)

            # --- phase C: v projection ---
            vaccs = [acc_p.tile([P, CH], FP32, tag="acc", name=f"vacc{j}")
                     for j in range(NJ)]
            w_sb = w_sbs["wv"]
            for dc in range(NDC):
                nc.tensor.matmul(warm[:, 0:256], ones[:], ones2[:],
                                 start=True, stop=True)
                xt = xs.tile([P, L], FP16, tag="xt", name="xt")
                nc.sync.dma_start(xt[:], vt_d[dc * P:(dc + 1) * P, :])
                for j in range(NJ):
                    nc.tensor.matmul(
                        vaccs[j][:], w_sb[:, dc * P:dc * P + P],
                        xt[:, j * CH:(j + 1) * CH],
                        start=(dc == 0), stop=(dc == NDC - 1))

            # finalize row sums: partition-reduce on gpsimd (idle engine),
            # reciprocal on DVE; deferred here so neither blocks the q/v
            # DMA-issue queues mid-stream
            from concourse import bass_isa
            for j in range(NJ):
                rs = rsum_all[:, j * CH:(j + 1) * CH]
                rt = rtot_all[:, j * CH:(j + 1) * CH]
                nc.gpsimd.partition_all_reduce(rt, rs, channels=P,
                                               reduce_op=bass_isa.ReduceOp.add)
                nc.vector.reciprocal_approx_fast(
                    rinv_all[:, j * CH:(j + 1) * CH], rt)

            # --- phase D: evict v, transpose, PV, normalize, Y, write ---
            def evict_v(j):
                # split psum->sbuf eviction across engines to cut latency
                h = CH // 2
                nc.vector.tensor_copy(vT[:, j * CH:j * CH + h],
                                      vaccs[j][:, 0:h])
                nc.scalar.copy(vT[:, j * CH + h:(j + 1) * CH],
                               vaccs[j][:, h:CH])

            def transposes(j):
                for i, c in enumerate(range(4 * j, 4 * j + 4)):
                    tp = rot_p.tile([P, P], FP16, tag="rot", name="tp")
                    nc.tensor.transpose(tp[:], vT[:, c * P:(c + 1) * P],
                                        ident[:])
                    if i % 2 == 0:
                        nc.vector.tensor_copy(v_nat[:, c * P:(c + 1) * P], tp[:])
                    else:
                        nc.scalar.copy(v_nat[:, c * P:(c + 1) * P], tp[:])

            def pv_chunk(j):
                ot = rot_p.tile([P, CH], FP32, tag="rot", name="ot")
                for c in range(4 * j + 4):
                    nc.tensor.matmul(ot[:], v_nat[:, c * P:(c + 1) * P],
                                     et_all[:, ET_OFF[(j, c)]:ET_OFF[(j, c)] + CH],
                                     start=(c == 0), stop=(c == 4 * j + 3))
                nc.vector.tensor_mul(oT[:, j * CH:(j + 1) * CH], ot[:],
                                     rinv_all[:, j * CH:(j + 1) * CH])

            def y_chunk(j):
                for t in range(CH // P):
                    lq0 = j * CH + t * P
                    yev = ev_pool.tile([P, D], YDT, tag="yev", name="yev")
                    for dch in range(D // CH):
                        yps = rot_p.tile([P, CH], FP32, tag="rot", name="yps")
                        nc.tensor.matmul(yps[:], oT[:, lq0:lq0 + P],
                                         wo_sb[:, dch * CH:(dch + 1) * CH],
                                         start=True, stop=True)
                        dst = yev[:, dch * CH:(dch + 1) * CH]
                        if dch % 2 == 0:
                            nc.vector.tensor_copy(dst, yps[:])
                        else:
                            nc.scalar.copy(dst, yps[:])
                    nc.sync.dma_start(y_d[lq0:lq0 + P, :], yev[:])

            evict_v(0)
            transposes(0)
            pv_chunk(0)
            evict_v(1)
            transposes(1)
            pv_chunk(1)
            y_chunk(0)
            evict_v(2)
            transposes(2)
            pv_chunk(2)
            y_chunk(1)
            evict_v(3)
            transposes(3)
            pv_chunk(3)
            y_chunk(2)
            y_chunk(3)
    nc.compile()
    return nc


_NC = None


def _get_nc():
    global _NC
    if _NC is None:
        _NC = _build()
    return _NC


def _pack_w(w):
    """(D, dk) fp32 -> SBUF image (128, NDC*dk): out[p, dc*dk+m] = w[dc*128+p, m]"""
    return np.ascontiguousarray(
        w.reshape(NDC, P, -1).transpose(1, 0, 2).reshape(P, -1)).astype(np.float16)


def _make_in_maps(Q, K, V, Wq, Wk, Wv, Wo):
    f16 = np.float16
    # fold GQA group sum into Wq: head = g*KV_HEADS + h
    Wq_eff = np.asarray(Wq, np.float32).reshape(D, GROUPS, KV_HEADS, DK).sum(axis=1)
    acts = {}
    for b in range(B):
        qt = np.ascontiguousarray(np.asarray(Q[b], np.float32).T).astype(f16)
        # block qt by query chunk: (D, NJ, CH) -> (NJ*D, CH) contiguous
        qtb = np.ascontiguousarray(
            qt.reshape(D, NJ, CH).transpose(1, 0, 2).reshape(NJ * D, CH))
        acts[b] = {
            "qt": qtb,
            "kt": np.ascontiguousarray(np.asarray(K[b], np.float32).T).astype(f16),
            "vt": np.ascontiguousarray(np.asarray(V[b], np.float32).T).astype(f16),
        }
    Wk32, Wv32 = np.asarray(Wk, np.float32), np.asarray(Wv, np.float32)
    Wo32 = np.asarray(Wo, np.float32)
    in_maps = []
    for c in range(8):
        b, h = divmod(c, KV_HEADS)
        in_maps.append({
            **acts[b],
            "wq": _pack_w(Wq_eff[:, h, :]),
            "wk": _pack_w(Wk32[:, h * DK:(h + 1) * DK]),
            "wv": _pack_w(Wv32[:, h * DV:(h + 1) * DV]),
            "wo": Wo32[h * DV:(h + 1) * DV, :].astype(f16),
        })
    return in_maps


def _gather(results):
    Y = np.zeros((B, L, D), np.float32)
    for c in range(8):
        Y[c // KV_HEADS] += results[c]["y"].astype(np.float32)
    return Y


def kernel(Q, K, V, Wq, Wk, Wv, Wo):
    nc = _get_nc()
    in_maps = _make_in_maps(Q, K, V, Wq, Wk, Wv, Wo)
    res = bass_utils.run_bass_kernel_spmd(nc, in_maps, core_ids=list(range(8)))
    return _gather(res.results)


def _install_ntff_hook():
    """The agent image's antenv lacks axon_hooks; synthesize it so
    trace=True can reach the NTFF profiler in libaxon_pjrt.so."""
    import types
    import antenv
    if hasattr(antenv, "axon_hooks"):
        return
    mod = types.ModuleType("antenv.axon_hooks")
    _h = [None]
    mod.set_axon_ntff_profile_hook = lambda h: _h.__setitem__(0, h)
    mod.get_axon_ntff_profile_hook = lambda: _h[0]
    sys.modules["antenv.axon_hooks"] = mod
    antenv.axon_hooks = mod
    from trn_agent_boot.trn_boot import _ntff_profile_via_ctypes
    mod.set_axon_ntff_profile_hook(_ntff_profile_via_ctypes("/opt/axon/libaxon_pjrt.so"))


def kernel_traced(Q, K, V, Wq, Wk, Wv, Wo):
    """Like kernel() but profiles; returns (output, BassKernelResults)."""
    _install_ntff_hook()
    nc = _get_nc()
    in_maps = _make_in_maps(Q, K, V, Wq, Wk, Wv, Wo)
    res = bass_utils.run_bass_kernel_spmd(nc, in_maps, core_ids=list(range(8)),
                                          trace=True)
    return _gather(res.results), res
